# revision 12
# baseline (speedup 1.0000x reference)
"""Data-parallel MoE kernel for one TRN2 chip (8 NeuronCores), ZERO collectives.

Each core owns a 1024-token slice of x and computes its output slice fully:
  - gating in exact fp32 over the slice (routing matches the reference),
  - positions/gather-list: tokens compacted per expert into one concatenated
    segment list (uniform per-expert cap, 64-aligned),
  - FFN: for each of the 8 experts, stream that expert's host-pretransposed
    bf16 w1/w2 from HBM; weight-norm scales rc = g/max(||v||,eps) are computed
    on device from bf16 squares; rc1 folds into the Silu activation scale,
    rc2/b2 fold into the PSUM->SBUF output copy,
  - outputs scatter-add (bf16) into a local [TSL,D] buffer; a final dram->dram
    cast DMA emits the f32 output slice.

No cross-core traffic at all -> avoids the ~10ms fixed collective penalty
measured on this runtime.
"""

import numpy as np

import concourse.bass as bass
import concourse.mybir as mybir
import concourse.tile as tile
from concourse import bacc

F32 = mybir.dt.float32
BF16 = mybir.dt.bfloat16
I16 = mybir.dt.int16

AX = mybir.AxisListType
OP = mybir.AluOpType
ACT = mybir.ActivationFunctionType


class Cfg:
    def __init__(self, T=8192, D=1024, H=4096, E=8, NCORES=8):
        self.T, self.D, self.H, self.E = T, D, H, E
        self.NCORES = NCORES
        self.TSL = T // NCORES       # tokens per core
        self.NCH = self.TSL // 128   # 128-token chunks per slice (8)
        self.ND = D // 128
        self.NH = H // 128
        self.DUMP = 64
        self.QH = H // 4             # w1 quarter: 1024 columns of H
        self.QC = self.NH // 4       # w2 quarter: 8 h-chunks


def build_moe(nc, cfg: Cfg, cap, windows, capg):
    T, D, H, E = cfg.T, cfg.D, cfg.H, cfg.E
    TSL, NCH, ND, NH, DUMP = cfg.TSL, cfg.NCH, cfg.ND, cfg.NH, cfg.DUMP
    QH, QC = cfg.QH, cfg.QC
    TOT = cap * E
    NBJ = TOT // 128
    NTB = (capg + 127) // 128
    NSB = (cap + 127) // 128   # scatter slot blocks (og partition-blocks)
    assert cap % 128 == 0 and cap <= 512 and capg <= cap and capg % 64 == 0
    GE = NCH * E  # 64 flat (chunk, expert) columns

    # ---------------- kernel I/O ----------------
    xsT = nc.dram_tensor("xsT", [128, ND * TSL], F32, kind="ExternalInput").ap()
    xbf = nc.dram_tensor("xbf", [TSL + DUMP, D], BF16, kind="ExternalInput").ap()
    gvT = nc.dram_tensor("gvT", [128, ND * E], F32, kind="ExternalInput").ap()
    gateg = nc.dram_tensor("gateg", [1, E], F32, kind="ExternalInput").ap()
    gateb = nc.dram_tensor("gateb", [1, E], F32, kind="ExternalInput").ap()
    w1t = nc.dram_tensor("w1t", [E * 128, ND * H], BF16, kind="ExternalInput").ap()
    w2t = nc.dram_tensor("w2t", [E * 128, NH * D], BF16, kind="ExternalInput").ap()
    g1w = nc.dram_tensor("g1w", [128, E * NH], F32, kind="ExternalInput").ap()
    b1w = nc.dram_tensor("b1w", [128, E * NH], F32, kind="ExternalInput").ap()
    g2r = nc.dram_tensor("g2r", [E, D], F32, kind="ExternalInput").ap()
    b2r = nc.dram_tensor("b2r", [E, D], BF16, kind="ExternalInput").ap()
    u128 = nc.dram_tensor("u128", [128, 128], F32, kind="ExternalInput").ap()
    u64be = nc.dram_tensor("u64be", [GE, GE], F32, kind="ExternalInput").ap()
    segrow = nc.dram_tensor("segrow", [1, GE], F32, kind="ExternalInput").ap()
    ones1 = nc.dram_tensor("ones1", [1, 128], F32, kind="ExternalInput").ap()
    ones1b = nc.dram_tensor("ones1b", [1, 128], BF16, kind="ExternalInput").ap()
    onescb = nc.dram_tensor("onescb", [128, 1], BF16, kind="ExternalInput").ap()
    onescf = nc.dram_tensor("onescf", [128, 1], F32, kind="ExternalInput").ap()
    ident = nc.dram_tensor("ident", [128, 128], F32, kind="ExternalInput").ap()
    tvals = nc.dram_tensor("tvals", [128, NCH], F32, kind="ExternalInput").ap()
    jgrid = nc.dram_tensor("jgrid", [128, 128], F32, kind="ExternalInput").ap()
    dumpo = nc.dram_tensor("dumpo", [128, 1], F32, kind="ExternalInput").ap()
    out_ext = nc.dram_tensor("out", [TSL, D], F32, kind="ExternalOutput").ap()

    # ---------------- internal DRAM ----------------
    obuf = nc.dram_tensor("obuf", [TSL + DUMP, D], BF16).ap()

    w1v = w1t.rearrange("(e p) (d h) -> e p d h", e=E, d=ND)
    w2v = w2t.rearrange("(e p) (c d) -> e p c d", e=E, c=NH)
    xsTv = xsT.rearrange("p (d t) -> p d t", d=ND)

    with tile.TileContext(nc) as tc:
        with (
            tc.tile_pool(name="consts", bufs=1) as cpool,
            tc.tile_pool(name="psS", bufs=2, space="PSUM") as psS,
            tc.tile_pool(name="psW", bufs=1, space="PSUM") as psW,
            tc.tile_pool(name="psPH", bufs=2, space="PSUM") as psPH,
            tc.tile_pool(name="psPO", bufs=3, space="PSUM") as psPO,
        ):
            # ---- constants ----
            u128_sb = cpool.tile([128, 128], F32)
            nc.scalar.dma_start(u128_sb[:], u128)
            u64_sb = cpool.tile([GE, GE], F32)
            nc.scalar.dma_start(u64_sb[:], u64be)
            seg_sb = cpool.tile([1, GE], F32)
            nc.scalar.dma_start(seg_sb[:], segrow)
            ones1_sb = cpool.tile([1, 128], F32)
            nc.scalar.dma_start(ones1_sb[:], ones1)
            ones1b_sb = cpool.tile([1, 128], BF16)
            nc.scalar.dma_start(ones1b_sb[:], ones1b)
            onescb_sb = cpool.tile([128, 1], BF16)
            nc.scalar.dma_start(onescb_sb[:], onescb)
            onescf_sb = cpool.tile([128, 1], F32)
            nc.scalar.dma_start(onescf_sb[:], onescf)
            id_sb = cpool.tile([128, 128], F32)
            nc.scalar.dma_start(id_sb[:], ident)
            tv_sb = cpool.tile([128, NCH], F32)
            nc.scalar.dma_start(tv_sb[:], tvals)
            jg_sb = cpool.tile([128, 128], F32)
            nc.scalar.dma_start(jg_sb[:], jgrid)
            dumpo_sb = cpool.tile([128, 1], F32)
            nc.scalar.dma_start(dumpo_sb[:], dumpo)
            gvT_sb = cpool.tile([128, ND, E], F32)
            nc.scalar.dma_start(gvT_sb[:], gvT.rearrange("p (d e) -> p d e", d=ND))
            gg_sb = cpool.tile([1, E], F32)
            nc.scalar.dma_start(gg_sb[:], gateg)
            gb_sb = cpool.tile([1, E], F32)
            nc.scalar.dma_start(gb_sb[:], gateb)
            g1w_sb = cpool.tile([128, E * NH], F32)
            nc.sync.dma_start(g1w_sb[:], g1w)
            b1w_sb = cpool.tile([128, E * NH], F32)
            nc.sync.dma_start(b1w_sb[:], b1w)
            mask_grid = cpool.tile([128, NCH, E], F32)
            glw = cpool.tile([128, TOT // 16], I16)

            # ---- zero obuf (independent; overlaps everything) ----
            zt = cpool.tile([128, D], BF16)
            nc.gpsimd.memset(zt[:], 0.0)
            nfull = (TSL + DUMP) // 128
            for j in range(nfull):
                nc.sync.dma_start(obuf[j * 128:(j + 1) * 128, :], zt[:])
            rem = (TSL + DUMP) - nfull * 128
            if rem:
                nc.sync.dma_start(obuf[nfull * 128:TSL + DUMP, :], zt[:rem, :])

            # ---- gating (exact fp32) ----
            with tc.tile_pool(name="gat", bufs=2) as gp:
                sqg = gp.tile([128, ND, E], F32, tag="sqg", bufs=1)
                nc.vector.tensor_tensor(sqg[:], gvT_sb[:], gvT_sb[:], op=OP.mult)
                psg = psS.tile([128, 512], F32, tag="s")
                for dc in range(ND):
                    nc.tensor.matmul(psg[:1, :E], lhsT=onescf_sb[:],
                                     rhs=sqg[:, dc, :],
                                     start=(dc == 0), stop=(dc == ND - 1))
                grc = gp.tile([1, E], F32, tag="grc", bufs=1)
                nc.vector.tensor_copy(grc[:], psg[:1, :E])
                nc.scalar.sqrt(grc[:], grc[:])
                nc.vector.tensor_scalar_max(grc[:], grc[:], 1e-12)
                nc.vector.reciprocal(grc[:], grc[:])
                nc.vector.tensor_tensor(grc[:], grc[:], gg_sb[:], op=OP.mult)
                pbg = psS.tile([128, 512], F32, tag="s")
                nc.tensor.matmul(pbg[:, :E], lhsT=ones1_sb[:], rhs=grc[:],
                                 start=True, stop=True)
                gwT = gp.tile([128, ND, E], F32, tag="gwT", bufs=1)
                for dc in range(ND):
                    nc.vector.tensor_tensor(gwT[:, dc, :], gvT_sb[:, dc, :],
                                            pbg[:, :E], op=OP.mult)

                for g in range(NCH):
                    xsb = gp.tile([128, ND, 128], F32, tag="xsb", bufs=2)
                    nc.scalar.dma_start(xsb[:], xsTv[:, :, g * 128:(g + 1) * 128])
                    pg = psS.tile([128, 512], F32, tag="s")
                    for dc in range(ND):
                        nc.tensor.matmul(pg[:, :E], lhsT=xsb[:, dc, :],
                                         rhs=gwT[:, dc, :],
                                         start=(dc == 0), stop=False)
                    nc.tensor.matmul(pg[:, :E], lhsT=ones1_sb[:], rhs=gb_sb[:],
                                     start=False, stop=True)
                    lg = gp.tile([128, E], F32, tag="lg", bufs=2)
                    nc.vector.tensor_copy(lg[:], pg[:, :E])
                    mx1 = gp.tile([128, 1], F32, tag="mx1", bufs=2)
                    nc.vector.tensor_reduce(mx1[:], lg[:], axis=AX.X, op=OP.max)
                    eq = gp.tile([128, E], F32, tag="eq", bufs=2)
                    nc.vector.tensor_tensor(eq[:], lg[:],
                                            mx1[:].to_broadcast([128, E]),
                                            op=OP.is_equal)
                    nc.vector.tensor_scalar_mul(eq[:], eq[:], 1e30)
                    nc.vector.tensor_tensor(eq[:], lg[:], eq[:], op=OP.subtract)
                    mx2 = gp.tile([128, 1], F32, tag="mx2", bufs=2)
                    nc.vector.tensor_reduce(mx2[:], eq[:], axis=AX.X, op=OP.max)
                    nc.vector.tensor_tensor(mask_grid[:, g, :], lg[:],
                                            mx2[:].to_broadcast([128, E]),
                                            op=OP.is_ge)

            # ---- positions + gather list ----
            with tc.tile_pool(name="pos", bufs=2) as qp:
                mg_flat = mask_grid[:].rearrange("p g e -> p (g e)")
                ppos = psS.tile([128, 512], F32, tag="s")
                nc.tensor.matmul(ppos[:, :GE], lhsT=u128_sb[:], rhs=mg_flat,
                                 start=True, stop=True)
                pref = qp.tile([128, GE], F32, tag="pref", bufs=1)
                nc.vector.tensor_copy(pref[:], ppos[:, :GE])
                ptc = psS.tile([128, 512], F32, tag="s")
                nc.tensor.transpose(ptc[:GE, :128], pref[:], id_sb[:])
                totc = qp.tile([GE, 1], F32, tag="totc", bufs=1)
                nc.vector.tensor_copy(totc[:], ptc[:GE, 127:128])
                poff = psS.tile([128, 512], F32, tag="s")
                nc.tensor.matmul(poff[:1, :GE], lhsT=totc[:], rhs=u64_sb[:],
                                 start=True, stop=True)
                offs = qp.tile([1, GE], F32, tag="offs", bufs=1)
                nc.vector.tensor_copy(offs[:], poff[:1, :GE])
                nc.vector.tensor_tensor(offs[:], offs[:], seg_sb[:], op=OP.add)
                pbc = psS.tile([128, 512], F32, tag="s")
                nc.tensor.matmul(pbc[:, :GE], lhsT=ones1_sb[:], rhs=offs[:],
                                 start=True, stop=True)
                pos = qp.tile([128, GE], F32, tag="pos", bufs=1)
                nc.vector.tensor_tensor(pos[:], pref[:], mg_flat,
                                        op=OP.subtract)
                nc.vector.tensor_tensor(pos[:], pos[:], pbc[:, :GE], op=OP.add)
                nc.vector.tensor_scalar_add(pos[:], pos[:], 1.0e6)
                nc.vector.tensor_tensor(pos[:], pos[:], mg_flat, op=OP.mult)
                nc.vector.tensor_scalar_add(pos[:], pos[:], -1.0e6)

                pgl = psS.tile([128, 512], F32, tag="s")
                for J in range(NBJ):
                    cand = windows[J]
                    jgJ = qp.tile([128, 128], F32, tag="jgJ", bufs=2)
                    nc.vector.tensor_scalar_add(jgJ[:], jg_sb[:], float(128 * J))
                    for k, (c, g) in enumerate(cand):
                        oh = qp.tile([128, 128], F32, tag="oh", bufs=4)
                        nc.vector.tensor_tensor(
                            oh[:], pos[:, c:c + 1].to_broadcast([128, 128]),
                            jgJ[:], op=OP.is_equal)
                        nc.tensor.matmul(pgl[:, J:J + 1], lhsT=oh[:],
                                         rhs=tv_sb[:, g:g + 1],
                                         start=(k == 0), stop=(k == len(cand) - 1))
                gl = qp.tile([128, NBJ], F32, tag="gl", bufs=1)
                nc.vector.tensor_copy(gl[:], pgl[:, :NBJ])
                eqz = qp.tile([128, NBJ], F32, tag="eqz", bufs=1)
                nc.vector.tensor_scalar(eqz[:], gl[:], 0.0, None, op0=OP.is_equal)
                nc.vector.tensor_tensor(eqz[:], eqz[:],
                                        dumpo_sb[:].to_broadcast([128, NBJ]),
                                        op=OP.mult)
                nc.vector.tensor_tensor(gl[:], gl[:], eqz[:], op=OP.add)
                nc.vector.tensor_scalar_add(gl[:], gl[:], -1.0)

                # wrap: [p, J] -> [q, (J, ph)] with p = ph*16+q
                pT = psS.tile([128, 512], F32, tag="s")
                nc.tensor.transpose(pT[:NBJ, :128], gl[:], id_sb[:])
                glTs = qp.tile([NBJ, 128], F32, tag="glTs", bufs=1)
                nc.vector.tensor_copy(glTs[:], pT[:NBJ, :128])
                glwf = qp.tile([16, NBJ, 8], F32, tag="glwf", bufs=1)
                for ph in range(8):
                    pq = psS.tile([128, 512], F32, tag="s")
                    nc.tensor.transpose(pq[:16, :NBJ],
                                        glTs[:, ph * 16:(ph + 1) * 16],
                                        id_sb[:NBJ, :NBJ])
                    nc.vector.tensor_copy(glwf[:, :, ph], pq[:16, :NBJ])
                glw16 = qp.tile([16, TOT // 16], I16, tag="glw16", bufs=1)
                nc.vector.tensor_copy(glw16[:],
                                      glwf[:].rearrange("q g h -> q (g h)"))
                nc.sync.dma_start(glw[0:16, :], glw16[:])
                nc.sync.dma_start(glw[16:32, :], glw[0:16, :])
                nc.sync.dma_start(glw[32:64, :], glw[0:32, :])
                nc.sync.dma_start(glw[64:128, :], glw[0:64, :])

            # ---- FFN over all experts, weights streamed ----
            with (
                tc.tile_pool(name="wq1", bufs=4) as wq1p,
                tc.tile_pool(name="wq2", bufs=2) as wq2p,
                tc.tile_pool(name="ffn", bufs=2) as fp,
            ):
                for e in range(E):
                    sgo = cap * e
                    glw_e = glw[:, sgo // 16:(sgo + cap) // 16]
                    xgt = fp.tile([128, ND, cap], BF16, tag="xgt", bufs=2)
                    nc.gpsimd.dma_gather(xgt[:], xbf, glw_e, cap, cap, D,
                                         transpose=True)

                    # -- w1 stream + column sumsq (accumulated wrapped) --
                    w1qs = []
                    rw1 = psW.tile([128, NH], F32, tag="w")
                    for q in range(4):
                        w1q = wq1p.tile([128, ND, QH], BF16, tag="w1q")
                        nc.sync.dma_start(w1q[:],
                                          w1v[e, :, :, q * QH:(q + 1) * QH])
                        w1qs.append(w1q)
                        for st in range(QH // 512):
                            ps = psS.tile([128, 512], F32, tag="s")
                            for dc in range(ND):
                                sqt = fp.tile([128, 512], BF16, tag="sqt",
                                              bufs=8)
                                nc.vector.tensor_tensor(
                                    sqt[:],
                                    w1q[:, dc, st * 512:(st + 1) * 512],
                                    w1q[:, dc, st * 512:(st + 1) * 512],
                                    op=OP.mult)
                                nc.tensor.matmul(ps[:1, :512],
                                                 lhsT=onescb_sb[:], rhs=sqt[:],
                                                 start=(dc == 0),
                                                 stop=(dc == ND - 1))
                            srow = fp.tile([1, 512], F32, tag="srow", bufs=2)
                            nc.vector.tensor_copy(srow[:], ps[:1, :512])
                            for k in range(4):
                                hcg = q * 8 + st * 4 + k
                                nc.tensor.transpose(
                                    rw1[:, hcg:hcg + 1],
                                    srow[:, k * 128:(k + 1) * 128],
                                    id_sb[:1, :1])
                    rc1 = fp.tile([128, NH], F32, tag="rc1", bufs=2)
                    nc.vector.tensor_copy(rc1[:], rw1[:])
                    nc.scalar.sqrt(rc1[:], rc1[:])
                    nc.vector.tensor_scalar_max(rc1[:], rc1[:], 1e-12)
                    nc.vector.reciprocal(rc1[:], rc1[:])
                    nc.vector.tensor_tensor(rc1[:], rc1[:],
                                            g1w_sb[:, e * NH:(e + 1) * NH],
                                            op=OP.mult)

                    # -- GEMM1 + silu(scale=rc1, bias=b1) --
                    hT = fp.tile([128, NH, capg], BF16, tag="hT", bufs=2)
                    for q in range(4):
                        for hc in range(8):
                            hcg = q * 8 + hc
                            ph = psPH.tile([128, capg], F32, tag="ph")
                            for dc in range(ND):
                                nc.tensor.matmul(
                                    ph[:],
                                    lhsT=w1qs[q][:, dc, hc * 128:(hc + 1) * 128],
                                    rhs=xgt[:, dc, :capg],
                                    start=(dc == 0), stop=(dc == ND - 1))
                            nc.scalar.activation(
                                hT[:, hcg, :], ph[:], ACT.Silu,
                                bias=b1w_sb[:, e * NH + hcg:e * NH + hcg + 1],
                                scale=rc1[:, hcg:hcg + 1])

                    # -- GEMM2 in two D-halves; w2 streamed per half, sumsq
                    #    piggybacks on the first half's stream --
                    ssq2 = fp.tile([1, D], F32, tag="ssq2", bufs=1)
                    rc2b = fp.tile([128, D], BF16, tag="rc2b", bufs=1)
                    b2b = fp.tile([128, D], BF16, tag="b2b", bufs=1)
                    og = fp.tile([128, NSB, D], BF16, tag="og", bufs=2)
                    for dn in range(2):
                        po_t = [psPO.tile([128, 512], F32, tag="po",
                                           name=f"po{e}_{dn}_{tb}")
                                for tb in range(NTB)]
                        for q in range(4):
                            w2q = wq2p.tile([128, QC, D], BF16, tag="w2q")
                            nc.sync.dma_start(
                                w2q[:], w2v[e, :, q * QC:(q + 1) * QC, :])
                            if dn == 0:
                                for st in range(D // 512):
                                    ps = psS.tile([128, 512], F32, tag="s")
                                    for hc in range(QC):
                                        sqt = fp.tile([128, 512], BF16,
                                                      tag="sqt", bufs=8)
                                        nc.vector.tensor_tensor(
                                            sqt[:],
                                            w2q[:, hc, st * 512:(st + 1) * 512],
                                            w2q[:, hc, st * 512:(st + 1) * 512],
                                            op=OP.mult)
                                        nc.tensor.matmul(
                                            ps[:1, :512], lhsT=onescb_sb[:],
                                            rhs=sqt[:], start=(hc == 0),
                                            stop=(hc == QC - 1))
                                    if q == 0:
                                        nc.vector.tensor_copy(
                                            ssq2[:, st * 512:(st + 1) * 512],
                                            ps[:1, :512])
                                    else:
                                        nc.vector.tensor_tensor(
                                            ssq2[:, st * 512:(st + 1) * 512],
                                            ssq2[:, st * 512:(st + 1) * 512],
                                            ps[:1, :512], op=OP.add)
                            for hc in range(QC):
                                hcg = q * QC + hc
                                for tb in range(NTB):
                                    tw = min(128, capg - tb * 128)
                                    nc.tensor.matmul(
                                        po_t[tb][:tw, :],
                                        lhsT=hT[:, hcg,
                                                tb * 128:tb * 128 + tw],
                                        rhs=w2q[:, hc,
                                                dn * 512:(dn + 1) * 512],
                                        start=(hcg == 0), stop=(hcg == NH - 1))
                        if dn == 0:
                            rc2 = fp.tile([1, D], F32, tag="rc2", bufs=1)
                            nc.vector.tensor_copy(rc2[:], ssq2[:])
                            nc.scalar.sqrt(rc2[:], rc2[:])
                            nc.vector.tensor_scalar_max(rc2[:], rc2[:], 1e-12)
                            nc.vector.reciprocal(rc2[:], rc2[:])
                            g2row = fp.tile([1, D], F32, tag="g2row",
                                            bufs=1)
                            nc.sync.dma_start(g2row[:], g2r[e:e + 1, :])
                            nc.vector.tensor_tensor(rc2[:], rc2[:], g2row[:],
                                                    op=OP.mult)
                            b2row = fp.tile([1, D], BF16, tag="b2row", bufs=1)
                            nc.sync.dma_start(b2row[:], b2r[e:e + 1, :])
                            for dd in range(2):
                                pb = psS.tile([128, 512], F32, tag="s")
                                nc.tensor.matmul(
                                    pb[:], lhsT=ones1_sb[:],
                                    rhs=rc2[:, dd * 512:(dd + 1) * 512],
                                    start=True, stop=True)
                                nc.vector.tensor_copy(
                                    rc2b[:, dd * 512:(dd + 1) * 512], pb[:])
                                pb2 = psS.tile([128, 512], F32, tag="s")
                                nc.tensor.matmul(
                                    pb2[:], lhsT=ones1b_sb[:],
                                    rhs=b2row[:, dd * 512:(dd + 1) * 512],
                                    start=True, stop=True)
                                nc.vector.tensor_copy(
                                    b2b[:, dd * 512:(dd + 1) * 512], pb2[:])
                        for tb in range(NTB):
                            tw = min(128, capg - tb * 128)
                            nc.vector.tensor_tensor(
                                og[:tw, tb, dn * 512:(dn + 1) * 512],
                                po_t[tb][:tw, :],
                                rc2b[:tw, dn * 512:(dn + 1) * 512], op=OP.mult)
                            nc.vector.tensor_tensor(
                                og[:tw, tb, dn * 512:(dn + 1) * 512],
                                og[:tw, tb, dn * 512:(dn + 1) * 512],
                                b2b[:tw, dn * 512:(dn + 1) * 512], op=OP.add)
                    nc.gpsimd.dma_scatter_add(obuf, og[:], glw_e, cap, cap, D)

            # ---- final cast to f32 output ----
            nc.gpsimd.dma_start(out=out_ext, in_=obuf[0:TSL, :])

    return nc


def compute_plan(cfg: Cfg, x, gate_v, gate_g, gate_b, margin_pos=32,
                 margin_cap=32):
    """Host-side routing analysis -> (cap, windows) compile parameters."""
    T, D, E = cfg.T, cfg.D, cfg.E
    NCH, TSL, NC = cfg.NCH, cfg.TSL, cfg.NCORES
    xf = np.asarray(x, np.float64).reshape(T, D)
    gv = np.asarray(gate_v, np.float64)
    gw = gv / np.maximum(np.sqrt((gv * gv).sum(-1, keepdims=True)), 1e-12)
    gw = gw * np.asarray(gate_g, np.float64).reshape(E, 1)
    logits = xf @ gw.T + np.asarray(gate_b, np.float64).reshape(1, E)
    part = np.argpartition(-logits, 2, axis=1)[:, :2]
    sel = np.zeros((T, E), bool)
    sel[np.arange(T), part[:, 0]] = True
    sel[np.arange(T), part[:, 1]] = True
    sel = sel.reshape(NC, NCH, 128, E)
    cg = sel.sum(2)                     # [NC, NCH, E] per-chunk counts
    cnt = cg.sum(1)                     # [NC, E]
    capg = int(cnt.max()) + margin_cap
    capg = ((capg + 63) // 64) * 64
    cap = ((capg + 127) // 128) * 128
    off = np.cumsum(cg, axis=1) - cg    # [NC, NCH, E] exclusive chunk offsets
    NBJ = cap * E // 128
    windows = []
    for J in range(NBJ):
        lo_j, hi_j = 128 * J, 128 * J + 128
        cand = set()
        for e in range(E):
            for g in range(NCH):
                lo = cap * e + int(off[:, g, e].min()) - margin_pos
                hi = cap * e + int((off[:, g, e] + cg[:, g, e]).max()) + margin_pos
                if lo < hi_j and hi > lo_j:
                    cand.add((g * E + e, g))
        windows.append(tuple(sorted(cand)))
    return int(cap), tuple(windows), int(capg)


# test.py compatibility alias
def compute_windows(cfg, x, gate_v, gate_g, gate_b, margin=None):
    return compute_plan(cfg, x, gate_v, gate_g, gate_b)


def make_in_maps(cfg: Cfg, x, gate_v, gate_g, gate_b, w1_v, w1_g, b1, w2_v,
                 w2_g, b2):
    import ml_dtypes

    T, D, H, E = cfg.T, cfg.D, cfg.H, cfg.E
    TSL, NCH, ND, NH, DUMP = cfg.TSL, cfg.NCH, cfg.ND, cfg.NH, cfg.DUMP
    GE = NCH * E
    f32 = np.float32
    bf16 = ml_dtypes.bfloat16
    xf = np.ascontiguousarray(np.asarray(x, f32).reshape(T, D))

    def wrapT(M, nch):
        # [R, C=nch*128] -> [128, nch*R]: M[r, c*128+p] lands at [p, c*R + r]
        R, C = M.shape
        assert C == nch * 128
        return np.ascontiguousarray(
            M.reshape(R, nch, 128).transpose(2, 1, 0).reshape(128, nch * R))

    def wrap_pc(v, nch):
        return np.ascontiguousarray(np.asarray(v, f32).reshape(nch, 128).T)

    w1t = np.empty((E * 128, ND * H), bf16)
    w2t = np.empty((E * 128, NH * D), bf16)
    for e in range(E):
        w1t[e * 128:(e + 1) * 128] = wrapT(np.asarray(w1_v[e], f32), ND)
        w2t[e * 128:(e + 1) * 128] = wrapT(np.asarray(w2_v[e], f32), NH)
    g1w = np.concatenate([wrap_pc(w1_g[e], NH) for e in range(E)], axis=1)
    b1w = np.concatenate([wrap_pc(b1[e], NH) for e in range(E)], axis=1)
    g2r = np.ascontiguousarray(np.asarray(w2_g, f32))
    b2r = np.ascontiguousarray(np.asarray(b2, f32)).astype(bf16)
    gvTw = wrapT(np.ascontiguousarray(np.asarray(gate_v, f32)), ND)

    u128 = np.triu(np.ones((128, 128), f32))             # k <= m
    u64be = np.zeros((GE, GE), f32)
    for gp_ in range(NCH):
        for ep in range(E):
            for g in range(NCH):
                if gp_ < g:
                    u64be[gp_ * E + ep, g * E + ep] = 1.0
    ones1 = np.ones((1, 128), f32)
    onescol = np.ones((128, 1), f32)
    ident = np.eye(128, dtype=f32)
    tvals = np.ascontiguousarray(
        (np.arange(TSL, dtype=np.int64).reshape(NCH, 128).T + 1).astype(f32))
    jgrid = np.tile(np.arange(128, dtype=f32), (128, 1))
    dumpo = (TSL + 1 + (np.arange(128) % DUMP)).astype(f32).reshape(128, 1)

    cap = compute_plan(cfg, x, gate_v, gate_g, gate_b)[0]
    segrow = np.zeros((1, GE), f32)
    for g in range(NCH):
        for e in range(E):
            segrow[0, g * E + e] = cap * e

    in_maps = []
    for i in range(cfg.NCORES):
        xs = xf[i * TSL:(i + 1) * TSL]
        xbf = np.zeros((TSL + DUMP, D), bf16)
        xbf[:TSL] = xs.astype(bf16)
        in_maps.append({
            "xsT": np.ascontiguousarray(
                xs.reshape(TSL, ND, 128).transpose(2, 1, 0).reshape(
                    128, ND * TSL)),
            "xbf": xbf,
            "gvT": gvTw,
            "gateg": np.ascontiguousarray(np.asarray(gate_g, f32).reshape(1, E)),
            "gateb": np.ascontiguousarray(np.asarray(gate_b, f32).reshape(1, E)),
            "w1t": w1t,
            "w2t": w2t,
            "g1w": g1w,
            "b1w": b1w,
            "g2r": g2r,
            "b2r": b2r,
            "u128": u128,
            "u64be": u64be,
            "segrow": segrow,
            "ones1": ones1,
            "ones1b": ones1.astype(bf16),
            "onescb": onescol.astype(bf16),
            "onescf": onescol,
            "ident": ident,
            "tvals": tvals,
            "jgrid": jgrid,
            "dumpo": dumpo,
        })
    return in_maps


_COMPILED = {}


def get_compiled(cfg: Cfg, plan):
    cap, windows, capg = plan
    key = (cfg.T, cfg.D, cfg.H, cfg.E, cap, windows, capg)
    if key not in _COMPILED:
        nc = bacc.Bacc("TRN2", target_bir_lowering=False, debug=False,
                       num_devices=cfg.NCORES)
        build_moe(nc, cfg, cap, windows, capg)
        nc.compile()
        _COMPILED[key] = nc
    return _COMPILED[key]


def kernel(x, gate_v, gate_g, gate_b, w1_v, w1_g, b1, w2_v, w2_g, b2):
    from concourse.bass_utils import run_bass_kernel_spmd

    cfg = Cfg()
    plan = compute_plan(cfg, x, gate_v, gate_g, gate_b)
    nc = get_compiled(cfg, plan)
    in_maps = make_in_maps(cfg, np.asarray(x), np.asarray(gate_v),
                           np.asarray(gate_g), np.asarray(gate_b),
                           np.asarray(w1_v), np.asarray(w1_g), np.asarray(b1),
                           np.asarray(w2_v), np.asarray(w2_g), np.asarray(b2))
    res = run_bass_kernel_spmd(nc, in_maps, core_ids=list(range(cfg.NCORES)))
    shards = [res.results[i]["out"] for i in range(cfg.NCORES)]
    out = np.concatenate(shards, axis=0).astype(np.float32)
    B, S_, D_ = x.shape
    return out.reshape(B, S_, D_)
